# revision 1
# baseline (speedup 1.0000x reference)
"""Trainium2 Bass kernel for nn_Attention_2216203124924 (sparse/varlen GQA attention).

Full computation:
  xq/xk/xv = x @ {wq,wk,wv}.T ; per-head RMSNorm(q,k) ; RoPE via
  rope_cache[positions] ; GQA repeat ; per-segment causal attention
  (segments from cu_seqlens) ; out @ wo.T

Distribution (8 NeuronCores, tensor-parallel over heads):
  core c owns q-heads [4c,4c+4) and kv-head c (GQA groups align),
  wo is row-sharded; each core emits a partial [2048,4096] output and the
  host sums the 8 partials.

On-device layout is "transposed" ([feature, seq]) throughout so the
contraction dim always sits on SBUF partitions:
  qT/kT/vT from weight-stationary projection matmuls, RMSNorm stats via
  ones-column matmul + matmul-broadcast of rsqrt row, RoPE as elementwise
  muls with host-gathered cos/sin (+ PE swap-half permutation), scores^T =
  kT_tile.T @ qT, probs via unnormalized exp (scores are O(1), max-sub
  unneeded) with compile-time segment mask plan, PV accumulated over key
  tiles in PSUM, normalization by matmul-broadcast reciprocal row, and the
  output projection from attnT tiles against wo^T.

All matmul operands are float32r (~13-bit mantissa, full PE rate).
The segment/causal structure from cu_seqlens and the rope gather by
positions are resolved on the host at build time; the NEFF is specialized
to them.
"""

import os
import sys

import numpy as np

for _p in ("/opt/trn_rl_repo",):
    if os.path.isdir(_p) and _p not in sys.path:
        sys.path.insert(0, _p)

S = 2048
D = 4096
HD = 128
HALF = 64
N_HEADS = 32
N_KV = 8
NCORES = 8
QH = N_HEADS // NCORES          # 4 q heads per core
NO = QH + 2                     # o-tiles per core in qkv projection: q0..q3, k, v
DT = D // 128                   # 32 contraction tiles
MC = S // 512                   # 4 m-chunks of 512
NT = S // 128                   # 16 key tiles
EPS = 1e-6
SCALE = HD ** -0.5

LAST_RESULT = None  # BassKernelResults of the most recent run (for test harness)


def _attention_plan(cu_seqlens):
    """Compile-time mask plan from cu_seqlens.

    Returns (plan, mask_pack):
      plan[mc] = list of (nt, w0, w1, mask_ops); w0/w1 are column offsets
      (multiples of 128, relative to the 512-wide m-chunk) of the contiguous
      valid window; mask_ops = [(j, kind, idx)] for 128-col subtiles needing
      a multiplicative 0/1 mask: kind 'tri' uses the shared causal triangle,
      kind 'host' uses mask_pack[:, idx*128:(idx+1)*128].
    """
    idx = np.arange(S)
    seg = np.searchsorted(np.asarray(cu_seqlens), idx, side="right") - 1
    mask_qk = (seg[:, None] == seg[None, :]) & (idx[:, None] >= idx[None, :])
    mask_t = mask_qk.T  # [n, m]

    plan = []
    tiles = []
    tile_ids = {}
    for mc in range(MC):
        entries = []
        for nt in range(NT):
            blk = mask_t[nt * 128:(nt + 1) * 128, mc * 512:(mc + 1) * 512]
            if not blk.any():
                continue
            js = [j for j in range(4) if blk[:, j * 128:(j + 1) * 128].any()]
            jlo, jhi = min(js), max(js)
            assert js == list(range(jlo, jhi + 1)), "valid window not contiguous"
            mops = []
            for j in range(jlo, jhi + 1):
                sub = blk[:, j * 128:(j + 1) * 128]
                if sub.all():
                    continue
                m0g = mc * 512 + j * 128
                n0g = nt * 128
                if m0g == n0g and np.array_equal(
                    sub, idx[:128][None, :] >= idx[:128][:, None]
                ):
                    mops.append((j, "tri", -1))
                else:
                    key = sub.tobytes()
                    if key not in tile_ids:
                        tile_ids[key] = len(tiles)
                        tiles.append(sub.astype(np.float32))
                    mops.append((j, "host", tile_ids[key]))
            entries.append((nt, jlo * 128, (jhi + 1) * 128, mops))
        assert entries, "every query row attends to at least itself"
        plan.append(entries)

    if tiles:
        mask_pack = np.concatenate(tiles, axis=1)
    else:
        mask_pack = np.zeros((128, 128), dtype=np.float32)
    return plan, np.ascontiguousarray(mask_pack)


def _build_graph(plan, n_mask_cols):
    import concourse.bass as bass  # noqa: PLC0415
    import concourse.mybir as mybir  # noqa: PLC0415
    import concourse.tile as tile  # noqa: PLC0415
    from concourse import bacc  # noqa: PLC0415
    from contextlib import ExitStack  # noqa: PLC0415

    f32 = mybir.dt.float32
    f32r = mybir.dt.float32r
    bf16 = mybir.dt.bfloat16
    AF = mybir.ActivationFunctionType

    nc = bacc.Bacc()
    xT_p = nc.declare_dram_parameter("xT", [D, S], bf16, isOutput=False)
    wqkv_p = nc.declare_dram_parameter("w_qkv", [128, NO * DT * 128], bf16, isOutput=False)
    wo_p = nc.declare_dram_parameter("w_o", [128, QH * D], bf16, isOutput=False)
    cs_p = nc.declare_dram_parameter("cs", [128, 4 * S], f32r, isOutput=False)
    mask_p = nc.declare_dram_parameter("mask_pack", [128, n_mask_cols], f32r, isOutput=False)
    consts_p = nc.declare_dram_parameter("consts", [128, 5 * 128], f32r, isOutput=False)
    constsb_p = nc.declare_dram_parameter("consts_bf", [128, 2 * 128 + n_mask_cols], bf16, isOutput=False)
    out_p = nc.declare_dram_parameter("out", [S, D], f32, isOutput=True)

    with tile.TileContext(nc) as tc, ExitStack() as ctx:
        const = ctx.enter_context(tc.tile_pool(name="const", bufs=1))
        persist = ctx.enter_context(tc.tile_pool(name="persist", bufs=1))

        consts = const.tile([128, 5 * 128], f32r)
        ones_col = consts[:, 0:1]
        ones_row = consts[0:1, 0:128]
        swp = consts[:, 128:256]        # swap-halves permutation
        ident = consts[:, 256:384]      # identity (for PE transpose)
        tri = consts[:, 384:512]        # causal triangle in [n, m]: 1 iff m >= n
        sca_row = consts[0:1, 512:640]  # all = HD**0.5 (divide-by folds the attn scale)

        constsb = const.tile([128, 2 * 128 + n_mask_cols], bf16)
        ones_col_bf = constsb[:, 0:1]
        tri_bf = constsb[:, 128:256]
        mask_bf = constsb[:, 256:]

        eps_col = const.tile([128, 1], f32)
        nc.gpsimd.memset(eps_col[:], EPS)

        # persistent activations: q0..q3, k, v in transposed [feat, seq] layout
        qkvT = [persist.tile([128, S], f32r, tag=f"qkvT{o}", name=f"qkvT{o}") for o in range(NO)]
        rsq_all = persist.tile([1, (QH + 1) * S], f32r)

        # ---------------- stage 1: qkv projection + rms stats ----------------
        with ExitStack() as s1:
            pw = s1.enter_context(tc.tile_pool(name="wqkv", bufs=1))
            px = s1.enter_context(tc.tile_pool(name="xstream", bufs=6))
            pq1 = s1.enter_context(tc.tile_pool(name="s1scratch", bufs=1))
            pq = s1.enter_context(tc.tile_pool(name="qkvpsum", bufs=1, space="PSUM"))
            pqs = s1.enter_context(tc.tile_pool(name="sspsum", bufs=2, space="PSUM"))

            w_sb = pw.tile([128, NO * DT * 128], bf16)
            wchunk = DT // 8 * NO * 128
            for wci in range(8):
                eng = nc.sync if wci == 0 else nc.gpsimd
                eng.dma_start(
                    w_sb[:, wci * wchunk:(wci + 1) * wchunk],
                    wqkv_p[:, wci * wchunk:(wci + 1) * wchunk],
                )
            nc.gpsimd.dma_start(consts[:], consts_p[:])
            nc.gpsimd.dma_start(constsb[:], constsb_p[:])

            for mc in range(MC):
                msl = slice(mc * 512, (mc + 1) * 512)
                accs = [pq.tile([128, 512], f32, tag=f"acc{o}", name=f"acc{o}") for o in range(NO)]
                for d in range(DT):
                    xt = px.tile([128, 512], bf16, tag="xt")
                    nc.sync.dma_start(xt[:], xT_p[d * 128:(d + 1) * 128, msl])
                    for o in range(NO):
                        woff = (d * NO + o) * 128
                        nc.tensor.matmul(
                            accs[o][:],
                            w_sb[:, woff:woff + 128],
                            xt[:],
                            start=(d == 0),
                            stop=(d == DT - 1),
                        )
                for o in range(NO):
                    nc.vector.tensor_copy(qkvT[o][:, msl], accs[o][:])
                for o in range(QH + 1):
                    sq = pq1.tile([128, 512], f32r, tag="sq", name="sq", bufs=1)
                    nc.vector.tensor_mul(sq[:], qkvT[o][:, msl], qkvT[o][:, msl])
                    ss = pqs.tile([1, 512], f32, tag="ss", name="ss", padded_shape=[128, 512])
                    nc.tensor.matmul(ss[:], ones_col, sq[:], start=True, stop=True)
                    nc.scalar.activation(
                        rsq_all[0:1, o * S + mc * 512: o * S + (mc + 1) * 512],
                        ss[:], AF.Sqrt, bias=eps_col[0:1, :], scale=1.0 / HD,
                    )

        # ---------------- stage 2: rope + attention, interleaved per head ----------------
        with ExitStack() as s2:
            p2 = s2.enter_context(tc.tile_pool(name="persist2", bufs=1))
            v_sb = p2.tile([128, S], bf16)
            qbf = [p2.tile([128, S], bf16, tag=f"qbf{o}", name=f"qbf{o}") for o in range(QH + 1)]
            attnT = [p2.tile([128, S], bf16, tag=f"attnT{h}", name=f"attnT{h}") for h in range(QH)]
            wo_sb = p2.tile([128, QH * D], bf16)
            nc.gpsimd.dma_start(wo_sb[:], wo_p[:])

            kT = qbf[QH]
            vT = qkvT[QH + 1]

            with ExitStack() as s2b:
                pcs = s2b.enter_context(tc.tile_pool(name="csstream", bufs=2))
                psc = s2b.enter_context(tc.tile_pool(name="s2scratch", bufs=2))
                pss = s2b.enter_context(tc.tile_pool(name="ssqpsum", bufs=2, space="PSUM"))
                psco = s2b.enter_context(tc.tile_pool(name="scpsum", bufs=2, space="PSUM"))
                pov = s2b.enter_context(tc.tile_pool(name="ovpsum", bufs=1, space="PSUM"))
                pden = s2b.enter_context(tc.tile_pool(name="denpsum", bufs=1, space="PSUM"))
                pyp = s2b.enter_context(tc.tile_pool(name="ypsum", bufs=2, space="PSUM"))
                pex = s2b.enter_context(tc.tile_pool(name="exsbuf", bufs=3))
                pnr = s2b.enter_context(tc.tile_pool(name="nrsbuf", bufs=2))
                pys = s2b.enter_context(tc.tile_pool(name="ysbuf", bufs=3))

                def rope_chain(o, mcs=range(MC)):
                    csb = 0 if o < QH else 2
                    row = sca_row if o < QH else ones_row
                    for mc in mcs:
                        msl = slice(mc * 512, (mc + 1) * 512)
                        cs1 = pcs.tile([128, 512], f32r, tag="cs1", name="cs1")
                        cs2 = pcs.tile([128, 512], f32r, tag="cs2", name="cs2")
                        nc.gpsimd.dma_start(cs1[:], cs_p[:, csb * S + mc * 512: csb * S + (mc + 1) * 512])
                        nc.gpsimd.dma_start(cs2[:], cs_p[:, (csb + 1) * S + mc * 512: (csb + 1) * S + (mc + 1) * 512])
                        bp = pss.tile([128, 512], f32, tag="ssbc", name="bp")
                        nc.tensor.matmul(bp[:], swp, qkvT[o][:, msl], start=True, stop=True)
                        t1 = psc.tile([128, 512], f32, tag="t1", name="t1")
                        nc.vector.tensor_mul(t1[:], qkvT[o][:, msl], cs1[:])
                        t2 = psc.tile([128, 512], f32, tag="t2", name="t2")
                        nc.vector.tensor_mul(t2[:], bp[:], cs2[:])
                        nc.vector.tensor_add(t1[:], t1[:], t2[:])
                        bc = pss.tile([128, 512], f32, tag="ssbc", name="bc")
                        nc.tensor.matmul(
                            bc[:], row,
                            rsq_all[0:1, o * S + mc * 512: o * S + (mc + 1) * 512],
                            start=True, stop=True,
                        )
                        rrb = psc.tile([128, 512], f32, tag="rrb", name="rrb")
                        nc.vector.reciprocal_approx_fast(out=rrb[:], in_=bc[:])
                        nc.vector.tensor_mul(qbf[o][:, msl], t1[:], rrb[:])

                # k first, then v transposes, then each q head followed by its attention
                rope_chain(QH)
                for nt in range(NT):
                    nsl = slice(nt * 128, (nt + 1) * 128)
                    tp = pss.tile([128, 128], f32, tag="ssbc", name="tp")
                    nc.tensor.transpose(
                        tp[:], vT[:, nsl].bitcast(mybir.dt.float32), ident.bitcast(mybir.dt.float32)
                    )
                    nc.vector.tensor_copy(v_sb[:, nsl], tp[:])

                for mc in range(MC):
                    for h in range(QH):
                        rope_chain(h, mcs=[mc])
                    for h in range(QH):
                        entries = plan[mc]
                        ov = pov.tile([128, 512], f32, tag="ov")
                        den = pden.tile([1, 512], f32, tag="den")
                        n_ent = len(entries)
                        for i, (nt, w0, w1, mops) in enumerate(entries):
                            nsl = slice(nt * 128, (nt + 1) * 128)
                            qsl = slice(mc * 512 + w0, mc * 512 + w1)
                            sc = psco.tile([128, 512], f32, tag="sc")
                            nc.tensor.matmul(
                                sc[:, w0:w1], kT[:, nsl], qbf[h][:, qsl],
                                start=True, stop=True,
                            )
                            ex = pex.tile([128, 512], bf16, tag="ex")
                            nc.scalar.activation(ex[:, w0:w1], sc[:, w0:w1], AF.Exp)
                            for (j, kind, tix) in mops:
                                jsl = slice(j * 128, (j + 1) * 128)
                                if kind == "tri":
                                    # zero strictly-below-diagonal (m < n) entries
                                    nc.gpsimd.affine_select(
                                        out=ex[:, jsl], in_=ex[:, jsl],
                                        compare_op=mybir.AluOpType.is_ge,
                                        fill=0.0, base=0,
                                        pattern=[[1, 128]], channel_multiplier=-1,
                                    )
                                else:
                                    nc.vector.tensor_mul(
                                        ex[:, jsl], ex[:, jsl],
                                        mask_bf[:, tix * 128:(tix + 1) * 128],
                                    )
                            first = i == 0
                            last = i == n_ent - 1
                            nc.tensor.matmul(
                                ov[:, w0:w1], v_sb[:, nsl], ex[:, w0:w1],
                                start=first, stop=last, skip_group_check=True,
                            )
                            nc.tensor.matmul(
                                den[0:1, w0:w1], ones_col_bf, ex[:, w0:w1],
                                start=first, stop=last, skip_group_check=True,
                            )
                        den_sb = pnr.tile([1, 512], f32r, tag="den_sb")
                        nc.vector.tensor_copy(den_sb[:], den[:])
                        bc = pss.tile([128, 512], f32, tag="ssbc", name="bc2")
                        nc.tensor.matmul(bc[:], ones_row, den_sb[:], start=True, stop=True)
                        bcs = pnr.tile([128, 512], f32, tag="bcs")
                        nc.vector.reciprocal_approx_fast(out=bcs[:], in_=bc[:])
                        nc.vector.tensor_mul(
                            attnT[h][:, mc * 512:(mc + 1) * 512], ov[:], bcs[:]
                        )

                    # output projection for this mc (fills PE while next mc's rope runs)
                    for j in range(4):
                        mt = mc * 4 + j
                        tsl = slice(mt * 128, (mt + 1) * 128)
                        for ec in range(D // 512):
                            yp = pyp.tile([128, 512], f32, tag="yp", name="yp")
                            for t in range(QH):
                                nc.tensor.matmul(
                                    yp[:],
                                    attnT[t][:, tsl],
                                    wo_sb[:, t * D + ec * 512: t * D + (ec + 1) * 512],
                                    start=(t == 0),
                                    stop=(t == QH - 1),
                                )
                            ys = pys.tile([128, 512], f32, tag="ys", name="ys")
                            if (j * (D // 512) + ec) % 2 == 0:
                                nc.scalar.activation(ys[:], yp[:], AF.Copy)
                            else:
                                nc.vector.tensor_copy(ys[:], yp[:])
                            nc.sync.dma_start(out_p[tsl, ec * 512:(ec + 1) * 512], ys[:])

    nc.finalize()
    return nc


def kernel(x, wq, wk, wv, wo, q_norm_w, k_norm_w, rope_cache, positions, cu_seqlens):
    global LAST_RESULT
    from concourse.bass_utils import run_bass_kernel_spmd  # noqa: PLC0415

    x = np.asarray(x, dtype=np.float32)
    wq = np.asarray(wq, dtype=np.float32)
    wk = np.asarray(wk, dtype=np.float32)
    wv = np.asarray(wv, dtype=np.float32)
    wo = np.asarray(wo, dtype=np.float32)
    q_norm_w = np.asarray(q_norm_w, dtype=np.float32)
    k_norm_w = np.asarray(k_norm_w, dtype=np.float32)
    rope_cache = np.asarray(rope_cache, dtype=np.float32)
    positions = np.asarray(positions)
    cu_seqlens = np.asarray(cu_seqlens)

    import ml_dtypes  # noqa: PLC0415

    # ---- host prep (shared) ----
    xT = np.ascontiguousarray(x[0].T.astype(ml_dtypes.bfloat16))  # [D, S]

    pos = positions.reshape(-1)
    cs = rope_cache[pos]               # [S, HALF, 2]
    cosT = cs[:, :, 0].T               # [HALF, S]
    sinT = cs[:, :, 1].T
    cs1 = np.concatenate([cosT, cosT], axis=0)    # [128, S]
    cs2 = np.concatenate([-sinT, sinT], axis=0)

    def fold(w):
        w = w.reshape(HD, 1)
        wsw = np.concatenate([w[HALF:], w[:HALF]], axis=0)
        return cs1 * w, cs2 * wsw

    cs1q, cs2q = fold(q_norm_w)
    cs1k, cs2k = fold(k_norm_w)
    cs_host = np.ascontiguousarray(
        np.concatenate([cs1q, cs2q, cs1k, cs2k], axis=1), dtype=np.float32
    )  # [128, 4S]

    plan, mask_pack = _attention_plan(cu_seqlens)

    consts_bf = np.zeros((128, 2 * 128 + mask_pack.shape[1]), dtype=np.float32)
    consts_bf[:, 0:128] = 1.0
    consts_bf[:, 128:256] = np.triu(np.ones((128, 128), dtype=np.float32))
    consts_bf[:, 256:] = mask_pack
    consts_bf = consts_bf.astype(ml_dtypes.bfloat16)

    consts = np.zeros((128, 5 * 128), dtype=np.float32)
    consts[:, 0:128] = 1.0
    swp = np.zeros((128, 128), dtype=np.float32)
    swp[np.arange(128), (np.arange(128) + HALF) % 128] = 1.0
    consts[:, 128:256] = swp
    consts[:, 256:384] = np.eye(128, dtype=np.float32)
    consts[:, 384:512] = np.triu(np.ones((128, 128), dtype=np.float32))
    consts[:, 512:640] = 1.0 / SCALE

    # ---- per-core weight shards ----
    in_maps = []
    for c in range(NCORES):
        w_all = np.concatenate(
            [
                wq[c * QH * HD:(c + 1) * QH * HD],   # [512, D]
                wk[c * HD:(c + 1) * HD],             # [128, D]
                wv[c * HD:(c + 1) * HD],             # [128, D]
            ],
            axis=0,
        )  # [NO*128, D]
        w_host = np.ascontiguousarray(
            w_all.reshape(NO, 128, DT, 128).transpose(3, 2, 0, 1)
            .reshape(128, NO * DT * 128).astype(ml_dtypes.bfloat16)
        )
        wo_c = wo[:, c * QH * HD:(c + 1) * QH * HD].T  # [512, D]
        wo_host = np.ascontiguousarray(
            wo_c.reshape(QH, 128, D).transpose(1, 0, 2)
            .reshape(128, QH * D).astype(ml_dtypes.bfloat16)
        )
        in_maps.append(
            {
                "xT": xT,
                "w_qkv": w_host,
                "w_o": wo_host,
                "cs": cs_host,
                "mask_pack": mask_pack,
                "consts": consts,
                "consts_bf": consts_bf,
            }
        )

    nc = _build_graph(plan, mask_pack.shape[1])
    res = run_bass_kernel_spmd(nc, in_maps, list(range(NCORES)))
    LAST_RESULT = res

    out = res.results[0]["out"].astype(np.float32)
    for c in range(1, NCORES):
        out = out + res.results[c]["out"]
    return out.reshape(1, S, D)



# revision 8
# speedup vs baseline: 1.2586x; 1.2586x over previous
"""Trainium2 Bass kernel for nn_Attention_2216203124924 (sparse/varlen GQA attention).

Full computation:
  xq/xk/xv = x @ {wq,wk,wv}.T ; per-head RMSNorm(q,k) ; RoPE via
  rope_cache[positions] ; GQA repeat ; per-segment causal attention
  (segments from cu_seqlens) ; out @ wo.T

Distribution (8 NeuronCores, tensor-parallel over heads):
  core c owns q-heads [4c,4c+4) and kv-head c (GQA groups align),
  wo is row-sharded; each core emits a partial output in transposed
  [D, S] bf16 layout and the host sums/transposes the 8 partials.

On-device layout is "transposed" ([feature, seq]) throughout so the
contraction dim always sits on SBUF partitions.  Schedule (v3):
  - stage 1: per m-chunk qkv projection (weight-stationary, 6 psum accs)
    with RMS stats, RoPE chains and V transposes for each m-chunk issued
    immediately after it so they ride under the next chunk's projection
    matmuls (psum: 6 accs + stats + rope = 8 banks).
  - stage 2: per m-chunk attention units followed immediately by that
    chunk's slice of the output projection — the outproj matmul stream
    (wo-stationary) fills the PE while the next chunk's attention chains
    (exp/mask/normalize) zigzag across ACT/DVE (psum: 2 sc + ov + den +
    aux + 3 yp = 8 banks).  outT [D, S] bf16 DMA'd per (of, mc) tile.

All matmul operands are bf16 (full PE rate — f32r compiles to the slow
fp32 path for the small broadcast matmuls).  The segment/causal structure
from cu_seqlens and the rope gather by positions are resolved on the host
at build time; the NEFF is specialized to them.
"""

import os
import sys

import numpy as np

for _p in ("/opt/trn_rl_repo",):
    if os.path.isdir(_p) and _p not in sys.path:
        sys.path.insert(0, _p)

S = 2048
D = 4096
HD = 128
HALF = 64
N_HEADS = 32
N_KV = 8
NCORES = 8
QH = N_HEADS // NCORES          # 4 q heads per core
NO = QH + 2                     # o-tiles per core in qkv projection: q0..q3, k, v
DT = D // 128                   # 32 contraction tiles
MC = S // 512                   # 4 m-chunks of 512
NT = S // 128                   # 16 key tiles
EPS = 1e-6
SCALE = HD ** -0.5

LAST_RESULT = None  # BassKernelResults of the most recent run (for test harness)


def _attention_plan(cu_seqlens):
    """Compile-time mask plan from cu_seqlens.

    Returns (plan, mask_pack):
      plan[mc] = list of (nt, w0, w1, mask_ops); w0/w1 are column offsets
      (multiples of 128, relative to the 512-wide m-chunk) of the contiguous
      valid window; mask_ops = [(j, kind, idx)] for 128-col subtiles needing
      a multiplicative 0/1 mask: kind 'tri' uses the shared causal triangle,
      kind 'host' uses mask_pack[:, idx*128:(idx+1)*128].
    """
    idx = np.arange(S)
    seg = np.searchsorted(np.asarray(cu_seqlens), idx, side="right") - 1
    mask_qk = (seg[:, None] == seg[None, :]) & (idx[:, None] >= idx[None, :])
    mask_t = mask_qk.T  # [n, m]

    plan = []
    tiles = []
    tile_ids = {}
    for mc in range(MC):
        entries = []
        for nt in range(NT):
            blk = mask_t[nt * 128:(nt + 1) * 128, mc * 512:(mc + 1) * 512]
            if not blk.any():
                continue
            js = [j for j in range(4) if blk[:, j * 128:(j + 1) * 128].any()]
            jlo, jhi = min(js), max(js)
            assert js == list(range(jlo, jhi + 1)), "valid window not contiguous"
            mops = []
            for j in range(jlo, jhi + 1):
                sub = blk[:, j * 128:(j + 1) * 128]
                if sub.all():
                    continue
                m0g = mc * 512 + j * 128
                n0g = nt * 128
                if m0g == n0g and np.array_equal(
                    sub, idx[:128][None, :] >= idx[:128][:, None]
                ):
                    mops.append((j, "tri", -1))
                else:
                    key = sub.tobytes()
                    if key not in tile_ids:
                        tile_ids[key] = len(tiles)
                        tiles.append(sub.astype(np.float32))
                    mops.append((j, "host", tile_ids[key]))
            entries.append((nt, jlo * 128, (jhi + 1) * 128, mops))
        assert entries, "every query row attends to at least itself"
        plan.append(entries)

    if tiles:
        mask_pack = np.concatenate(tiles, axis=1)
    else:
        mask_pack = np.zeros((128, 128), dtype=np.float32)
    return plan, np.ascontiguousarray(mask_pack)


def _build_graph(plan, n_mask_cols):
    import concourse.bass as bass  # noqa: PLC0415
    import concourse.mybir as mybir  # noqa: PLC0415
    import concourse.tile as tile  # noqa: PLC0415
    from concourse import bacc  # noqa: PLC0415
    from contextlib import ExitStack  # noqa: PLC0415

    f32 = mybir.dt.float32
    bf16 = mybir.dt.bfloat16
    AF = mybir.ActivationFunctionType

    nc = bacc.Bacc()
    xT_p = nc.declare_dram_parameter("xT", [D, S], bf16, isOutput=False)
    wqkv_p = nc.declare_dram_parameter("w_qkv", [128, NO * DT * 128], bf16, isOutput=False)
    wo_p = nc.declare_dram_parameter("w_o", [128, QH * D], bf16, isOutput=False)
    cs_p = nc.declare_dram_parameter("cs", [128, 4 * S], bf16, isOutput=False)
    constsb_p = nc.declare_dram_parameter("consts_bf", [128, 5 * 128 + n_mask_cols], bf16, isOutput=False)
    out_p = nc.declare_dram_parameter("outT", [D, S], bf16, isOutput=True)

    with tile.TileContext(nc) as tc, ExitStack() as ctx:
        const = ctx.enter_context(tc.tile_pool(name="const", bufs=1))
        persist = ctx.enter_context(tc.tile_pool(name="persist", bufs=1))
        prsq = ctx.enter_context(tc.tile_pool(name="rsq", bufs=2))

        constsb = const.tile([128, 5 * 128 + n_mask_cols], bf16)
        ones_col = constsb[:, 0:1]
        ones_row = constsb[0:1, 0:128]
        swp = constsb[:, 128:256]        # swap-halves permutation
        ident = constsb[:, 256:384]      # identity (for PE transpose)
        sca_row = constsb[0:1, 384:512]  # all = HD**0.5 (divide-by folds the attn scale)
        tri_bf = constsb[:, 512:640]     # [n, m] multiplicative causal mask: 1 iff m >= n
        mask_bf = constsb[:, 640:]

        eps_col = const.tile([128, 1], f32)
        nc.gpsimd.memset(eps_col[:], EPS)

        # persistent activations ([feature, seq] layouts)
        qkvT = [persist.tile([128, S], bf16, tag=f"qkvT{o}", name=f"qkvT{o}") for o in range(NO)]
        qbf = [persist.tile([128, S], bf16, tag=f"qbf{o}", name=f"qbf{o}") for o in range(QH + 1)]
        v_sb = persist.tile([128, S], bf16)
        attnT = [persist.tile([128, S], bf16, tag=f"attnT{h}", name=f"attnT{h}") for h in range(QH)]
        kT = qbf[QH]
        vT = qkvT[QH + 1]

        # ---------------- stage 1: qkv projection + stats + rope + v transpose ----------------
        with ExitStack() as s1:
            pw = s1.enter_context(tc.tile_pool(name="wqkv", bufs=1))
            px = s1.enter_context(tc.tile_pool(name="xstream", bufs=6))
            psq = s1.enter_context(tc.tile_pool(name="s1scratch", bufs=2))
            pcs = s1.enter_context(tc.tile_pool(name="csstream", bufs=2))
            psc1 = s1.enter_context(tc.tile_pool(name="ropescratch", bufs=2))
            pq = s1.enter_context(tc.tile_pool(name="qkvpsum", bufs=1, space="PSUM"))
            pqs = s1.enter_context(tc.tile_pool(name="sspsum", bufs=1, space="PSUM"))
            prp = s1.enter_context(tc.tile_pool(name="ropepsum", bufs=1, space="PSUM"))

            w_sb = pw.tile([128, NO * DT * 128], bf16)
            # startup-critical: first two d-tiles on the sync queue so the first
            # matmuls can begin within ~1us; the rest spread on other queues.
            dcols = NO * 128
            nc.sync.dma_start(w_sb[:, 0:2 * dcols], wqkv_p[:, 0:2 * dcols])
            nc.scalar.dma_start(constsb[:], constsb_p[:])
            for wci in range(6):
                c0 = (2 + 5 * wci) * dcols
                c1 = min((2 + 5 * (wci + 1)) * dcols, NO * DT * 128)
                nc.gpsimd.dma_start(w_sb[:, c0:c1], wqkv_p[:, c0:c1])

            rsq = {}

            def rope_chain(o, mc, cs1, cs2):
                """RoPE + per-token normalization for qkvT[o][:, mc] -> qbf[o]."""
                msl = slice(mc * 512, (mc + 1) * 512)
                row = sca_row if o < QH else ones_row
                bp = prp.tile([128, 512], f32, tag="rp", name="bp")
                nc.tensor.matmul(bp[:], swp, qkvT[o][:, msl], start=True, stop=True)
                t1 = psc1.tile([128, 512], bf16, tag="t1", name="t1")
                nc.vector.tensor_mul(t1[:], qkvT[o][:, msl], cs1[:])
                t2 = psc1.tile([128, 512], bf16, tag="t2", name="t2")
                nc.vector.tensor_mul(t2[:], bp[:], cs2[:])
                nc.vector.tensor_add(t1[:], t1[:], t2[:])
                bc = prp.tile([128, 512], f32, tag="rp", name="bc")
                nc.tensor.matmul(bc[:], row, rsq[(o, mc)][:], start=True, stop=True)
                rrb = psc1.tile([128, 512], f32, tag="rrb", name="rrb")
                nc.vector.reciprocal_approx_fast(out=rrb[:], in_=bc[:])
                nc.vector.tensor_mul(qbf[o][:, msl], t1[:], rrb[:])

            for mc in range(MC):
                msl = slice(mc * 512, (mc + 1) * 512)
                accs = [pq.tile([128, 512], f32, tag=f"acc{o}", name=f"acc{o}") for o in range(NO)]
                for d in range(DT):
                    xt = px.tile([128, 512], bf16, tag="xt")
                    nc.sync.dma_start(xt[:], xT_p[d * 128:(d + 1) * 128, msl])
                    for o in range(NO):
                        woff = (d * NO + o) * 128
                        nc.tensor.matmul(
                            accs[o][:],
                            w_sb[:, woff:woff + 128],
                            xt[:],
                            start=(d == 0),
                            stop=(d == DT - 1),
                        )
                for o in range(NO):
                    if o % 2 == 0:
                        nc.vector.tensor_copy(qkvT[o][:, msl], accs[o][:])
                    else:
                        nc.scalar.activation(qkvT[o][:, msl], accs[o][:], AF.Copy)
                for o in range(QH + 1):
                    sq = psq.tile([128, 512], bf16, tag="sq", name="sq")
                    nc.vector.tensor_mul(sq[:], qkvT[o][:, msl], qkvT[o][:, msl])
                    ss = pqs.tile([128, 512], f32, tag="ss", name="ss")
                    nc.tensor.matmul(ss[0:1, :], ones_col, sq[:], start=True, stop=True)
                    r = prsq.tile([1, 512], bf16, tag=f"rsq{o}", name=f"rsq{o}")
                    rsq[(o, mc)] = r
                    nc.scalar.activation(
                        r[:], ss[0:1, :], AF.Sqrt, bias=eps_col[0:1, :], scale=1.0 / HD,
                    )

                # rope + v-transpose for this mc ride under the next mc's projection
                cs1k = pcs.tile([128, 512], bf16, tag="cs1k", name="cs1k")
                cs2k = pcs.tile([128, 512], bf16, tag="cs2k", name="cs2k")
                nc.gpsimd.dma_start(cs1k[:], cs_p[:, 2 * S + mc * 512: 2 * S + (mc + 1) * 512])
                nc.gpsimd.dma_start(cs2k[:], cs_p[:, 3 * S + mc * 512: 3 * S + (mc + 1) * 512])
                rope_chain(QH, mc, cs1k, cs2k)
                cs1q = pcs.tile([128, 512], bf16, tag="cs1q", name="cs1q")
                cs2q = pcs.tile([128, 512], bf16, tag="cs2q", name="cs2q")
                nc.gpsimd.dma_start(cs1q[:], cs_p[:, 0 * S + mc * 512: 0 * S + (mc + 1) * 512])
                nc.gpsimd.dma_start(cs2q[:], cs_p[:, 1 * S + mc * 512: 1 * S + (mc + 1) * 512])
                for h in range(QH):
                    rope_chain(h, mc, cs1q, cs2q)
                for ntl in range(4):
                    nt = mc * 4 + ntl
                    nsl = slice(nt * 128, (nt + 1) * 128)
                    tp = prp.tile([128, 256], bf16, tag="rp", name="tp")
                    nc.tensor.transpose(tp[:, 0:128], vT[:, nsl], ident)
                    nc.vector.tensor_copy(v_sb[:, nsl], tp[:, 0:128])

        # ---------------- stage 2: attention + output projection, per m-chunk ----------------
        with ExitStack() as s2:
            pwo = s2.enter_context(tc.tile_pool(name="wopool", bufs=1))
            psco = s2.enter_context(tc.tile_pool(name="scpsum", bufs=2, space="PSUM"))
            pov = s2.enter_context(tc.tile_pool(name="ovpsum", bufs=1, space="PSUM"))
            pden = s2.enter_context(tc.tile_pool(name="denpsum", bufs=1, space="PSUM"))
            paux = s2.enter_context(tc.tile_pool(name="auxpsum", bufs=1, space="PSUM"))
            pyp = s2.enter_context(tc.tile_pool(name="ypsum", bufs=3, space="PSUM"))
            pex = s2.enter_context(tc.tile_pool(name="exsbuf", bufs=4))
            pnr = s2.enter_context(tc.tile_pool(name="nrsbuf", bufs=2))
            pys = s2.enter_context(tc.tile_pool(name="ysbuf", bufs=4))

            wo_sb = pwo.tile([128, QH * D], bf16)
            wchunk = QH * D // 4
            for wci in range(4):
                eng = [nc.gpsimd, nc.scalar, nc.gpsimd, nc.scalar][wci]
                eng.dma_start(
                    wo_sb[:, wci * wchunk:(wci + 1) * wchunk],
                    wo_p[:, wci * wchunk:(wci + 1) * wchunk],
                )

            for mc in range(MC):
                entries = plan[mc]
                n_ent = len(entries)
                msl = slice(mc * 512, (mc + 1) * 512)
                for h in range(QH):
                    ov = pov.tile([128, 512], f32, tag="ov")
                    den = pden.tile([128, 512], f32, tag="den")
                    for i, (nt, w0, w1, mops) in enumerate(entries):
                        nsl = slice(nt * 128, (nt + 1) * 128)
                        qsl = slice(mc * 512 + w0, mc * 512 + w1)
                        sc = psco.tile([128, 512], f32, tag="sc")
                        nc.tensor.matmul(
                            sc[:, w0:w1], kT[:, nsl], qbf[h][:, qsl],
                            start=True, stop=True,
                        )
                        ex = pex.tile([128, 512], bf16, tag="ex")
                        nc.scalar.activation(ex[:, w0:w1], sc[:, w0:w1], AF.Exp)
                        for (j, kind, tix) in mops:
                            jsl = slice(j * 128, (j + 1) * 128)
                            if kind == "tri":
                                nc.vector.tensor_mul(ex[:, jsl], ex[:, jsl], tri_bf)
                            else:
                                nc.vector.tensor_mul(
                                    ex[:, jsl], ex[:, jsl],
                                    mask_bf[:, tix * 128:(tix + 1) * 128],
                                )
                        first = i == 0
                        last = i == n_ent - 1
                        nc.tensor.matmul(
                            ov[:, w0:w1], v_sb[:, nsl], ex[:, w0:w1],
                            start=first, stop=last, skip_group_check=True,
                        )
                        nc.tensor.matmul(
                            den[0:1, w0:w1], ones_col, ex[:, w0:w1],
                            start=first, stop=last, skip_group_check=True,
                        )
                    den_sb = pnr.tile([1, 512], bf16, tag="den_sb")
                    nc.vector.tensor_copy(den_sb[:], den[0:1, :])
                    bc2 = paux.tile([128, 512], f32, tag="bc2")
                    nc.tensor.matmul(bc2[:], ones_row, den_sb[:], start=True, stop=True)
                    bcs = pnr.tile([128, 512], f32, tag="bcs")
                    nc.vector.reciprocal_approx_fast(out=bcs[:], in_=bc2[:])
                    nc.vector.tensor_mul(attnT[h][:, msl], ov[:], bcs[:])

                # this m-chunk's slice of the output projection: dense PE work
                # that covers the next chunk's attention chain latency
                for of in range(D // 128):
                    ofs = slice(of * 128, (of + 1) * 128)
                    yp = pyp.tile([128, 512], f32, tag="yp", name="yp")
                    for t in range(QH):
                        nc.tensor.matmul(
                            yp[:],
                            wo_sb[:, t * D + of * 128: t * D + (of + 1) * 128],
                            attnT[t][:, msl],
                            start=(t == 0),
                            stop=(t == QH - 1),
                        )
                    ys = pys.tile([128, 512], bf16, tag="ys", name="ys")
                    if of % 2 == 0:
                        nc.scalar.activation(ys[:], yp[:], AF.Copy)
                    else:
                        nc.vector.tensor_copy(ys[:], yp[:])
                    nc.sync.dma_start(out_p[ofs, msl], ys[:])

    nc.finalize()
    return nc


def kernel(x, wq, wk, wv, wo, q_norm_w, k_norm_w, rope_cache, positions, cu_seqlens):
    global LAST_RESULT
    from concourse.bass_utils import run_bass_kernel_spmd  # noqa: PLC0415

    x = np.asarray(x, dtype=np.float32)
    wq = np.asarray(wq, dtype=np.float32)
    wk = np.asarray(wk, dtype=np.float32)
    wv = np.asarray(wv, dtype=np.float32)
    wo = np.asarray(wo, dtype=np.float32)
    q_norm_w = np.asarray(q_norm_w, dtype=np.float32)
    k_norm_w = np.asarray(k_norm_w, dtype=np.float32)
    rope_cache = np.asarray(rope_cache, dtype=np.float32)
    positions = np.asarray(positions)
    cu_seqlens = np.asarray(cu_seqlens)

    import ml_dtypes  # noqa: PLC0415

    # ---- host prep (shared) ----
    xT = np.ascontiguousarray(x[0].T.astype(ml_dtypes.bfloat16))  # [D, S]

    pos = positions.reshape(-1)
    cs = rope_cache[pos]               # [S, HALF, 2]
    cosT = cs[:, :, 0].T               # [HALF, S]
    sinT = cs[:, :, 1].T
    cs1 = np.concatenate([cosT, cosT], axis=0)    # [128, S]
    cs2 = np.concatenate([-sinT, sinT], axis=0)

    def fold(w):
        w = w.reshape(HD, 1)
        wsw = np.concatenate([w[HALF:], w[:HALF]], axis=0)
        return cs1 * w, cs2 * wsw

    cs1q, cs2q = fold(q_norm_w)
    cs1k, cs2k = fold(k_norm_w)
    cs_host = np.ascontiguousarray(
        np.concatenate([cs1q, cs2q, cs1k, cs2k], axis=1)
    ).astype(ml_dtypes.bfloat16)  # [128, 4S]

    plan, mask_pack = _attention_plan(cu_seqlens)

    consts_bf = np.zeros((128, 5 * 128 + mask_pack.shape[1]), dtype=np.float32)
    consts_bf[:, 0:128] = 1.0
    swp = np.zeros((128, 128), dtype=np.float32)
    swp[np.arange(128), (np.arange(128) + HALF) % 128] = 1.0
    consts_bf[:, 128:256] = swp
    consts_bf[:, 256:384] = np.eye(128, dtype=np.float32)
    consts_bf[:, 384:512] = 1.0 / SCALE
    consts_bf[:, 512:640] = np.triu(np.ones((128, 128), dtype=np.float32))
    consts_bf[:, 640:] = mask_pack
    consts_bf = consts_bf.astype(ml_dtypes.bfloat16)

    # ---- per-core weight shards ----
    in_maps = []
    for c in range(NCORES):
        w_all = np.concatenate(
            [
                wq[c * QH * HD:(c + 1) * QH * HD],   # [512, D]
                wk[c * HD:(c + 1) * HD],             # [128, D]
                wv[c * HD:(c + 1) * HD],             # [128, D]
            ],
            axis=0,
        )  # [NO*128, D]
        w_host = np.ascontiguousarray(
            w_all.reshape(NO, 128, DT, 128).transpose(3, 2, 0, 1)
            .reshape(128, NO * DT * 128).astype(ml_dtypes.bfloat16)
        )
        wo_c = wo[:, c * QH * HD:(c + 1) * QH * HD].T  # [512, D]
        wo_host = np.ascontiguousarray(
            wo_c.reshape(QH, 128, D).transpose(1, 0, 2)
            .reshape(128, QH * D).astype(ml_dtypes.bfloat16)
        )
        in_maps.append(
            {
                "xT": xT,
                "w_qkv": w_host,
                "w_o": wo_host,
                "cs": cs_host,
                "consts_bf": consts_bf,
            }
        )

    nc = _build_graph(plan, mask_pack.shape[1])
    res = run_bass_kernel_spmd(nc, in_maps, list(range(NCORES)))
    LAST_RESULT = res

    out = res.results[0]["outT"].astype(np.float32)
    for c in range(1, NCORES):
        out = out + res.results[c]["outT"].astype(np.float32)
    return np.ascontiguousarray(out.T).reshape(1, S, D)
